# revision 13
# baseline (speedup 1.0000x reference)
"""Trainium2 Bass kernel: density-ratio estimator loss.

Math (from the reference):
    csum = sum_b c[b, l, :]                  # (L, C)
    v[l, :] = trans[l] @ csum[l]             # (L, Z)
    r[b, l] = z[b, l, :] . v[l, :]           # (B, L)
    out = exp(r)

Sharding across 8 NeuronCores (full inputs in, full output out):
    - c     : sharded along L (16 steps per core), host-transposed to
              [l, c, b] so csum is a free-axis reduce on the Scalar (ACT)
              engine yielding column-oriented csum directly.
    - trans : sharded along L, host-pre-transposed to [l, c, z] so PE
              matmuls produce v rows (l, z) directly.
    - v     : one tiny AllGather (16x256 f32 per rank -> 128x256).
    - z     : sharded along batch (256 rows per core); out shard is
              r^T (L, B/8).

Schedule: c (+tt) DMAs strictly before z DMAs so csum/v/AllGather fire as
early as possible; v matmuls overlap the c phase; z stream then feeds
DVE muls (+gpsimd for half) and DVE segmented reduces; one Exp; one
output DMA.
"""

import numpy as np

B, L, ZD, CD = 2048, 128, 256, 256
NCORES = 8
BP = B // NCORES  # 256 batches per core
LP = L // NCORES  # 16 steps per core
P = 128  # SBUF partitions

NB = 16  # batches per z tile
NZT = BP // NB  # 16 z tiles
C_BUFS = 3
Z_BUFS = 6

_PROGRAM = None


def _build_program():
    import concourse.bacc as bacc
    import concourse.mybir as mybir
    import concourse.tile as tile
    from concourse.tile_rust import add_dep_helper

    f32 = mybir.dt.float32
    nc = bacc.Bacc("TRN2", target_bir_lowering=False, debug=False,
                   num_devices=NCORES)

    z_p = nc.dram_tensor("z", [BP, L, ZD], f32, kind="ExternalInput").ap()
    ct_p = nc.dram_tensor("ct", [LP, CD, B], f32, kind="ExternalInput").ap()
    tt_p = nc.dram_tensor("tt", [LP, CD, ZD], f32, kind="ExternalInput").ap()
    out_p = nc.dram_tensor("out", [L, BP], f32, kind="ExternalOutput").ap()

    with tile.TileContext(nc) as tc:
        with (
            tc.tile_pool(name="cpool", bufs=C_BUFS) as cpool,
            tc.tile_pool(name="zpool", bufs=Z_BUFS) as zpool,
            tc.tile_pool(name="dummy", bufs=1) as dummy,
            tc.tile_pool(name="misc", bufs=1) as misc,
            tc.tile_pool(name="psum", bufs=1, space="PSUM") as psum,
            tc.tile_pool(name="dram", bufs=1, space="DRAM") as dram,
        ):
            # warmup collective: wake ncfw early so the real AllGather's
            # trigger latency is paid during the c phase
            wu_sb = misc.tile([1, 16], f32)
            wu_in = dram.tile([1, 16], f32)
            wu_out = dram.tile([NCORES, 16], f32)
            nc.gpsimd.memset(wu_sb[:], 0.0)
            nc.gpsimd.dma_start(wu_in[:], wu_sb[:])
            nc.gpsimd.collective_compute(
                "AllGather", mybir.AluOpType.bypass,
                replica_groups=[list(range(NCORES))],
                ins=[wu_in[:].opt()], outs=[wu_out[:].opt()],
            )

            # ---- phase C: csum columns via ACT free-axis reduction ---------
            # ct[l, c, b]; tile (128 c-half, 2048 b); accum -> csum column
            csum_sb = misc.tile([P, LP * 2], f32)
            tt_sb = misc.tile([P, LP, 2, ZD], f32)
            c_dmas = []
            dm = dummy.tile([P, 2 * B], f32)
            for l in range(LP):
                for h in range(2):
                    j = 2 * l + h
                    ctile = cpool.tile([P, B], f32, tag="c")
                    ring = nc.sync if j % 2 == 0 else nc.scalar
                    d = ring.dma_start(ctile[:], ct_p[l, h * P:(h + 1) * P, :])
                    c_dmas.append(d)
                    nc.scalar.activation(dm[:, 0:B], ctile[:],
                                         mybir.ActivationFunctionType.Copy,
                                         accum_out=csum_sb[:, j:j + 1])
                    if j == 3:
                        # transT early: v matmuls can then overlap the c phase
                        nc.sync.dma_start(
                            tt_sb[:],
                            tt_p.rearrange("l (h p) z -> p l h z", h=2))

            # ---- v rows: v[l, z] = sum_c csum[l, c] * transT[l, c, z] ------
            # PE out must start at partition 0 -> accumulate all v as one row
            pv = psum.tile([1, LP * ZD], f32, tag="ps")
            for l in range(LP):
                for h in range(2):
                    nc.tensor.matmul(
                        pv[0:1, l * ZD:(l + 1) * ZD],
                        csum_sb[:, 2 * l + h: 2 * l + h + 1],
                        tt_sb[:, l, h, :],
                        start=(h == 0), stop=(h == 1),
                    )
            v16_sb = dm[0:1, 0:LP * ZD]
            nc.scalar.copy(v16_sb, pv[:])

            # ---- AllGather v (gpsimd so the sync DMA FIFO never stalls) ----
            v16_dram = dram.tile([1, LP * ZD], f32)
            vfull_dram = dram.tile([L, ZD], f32)
            nc.gpsimd.dma_start(v16_dram[:], v16_sb)
            nc.gpsimd.collective_compute(
                "AllGather",
                mybir.AluOpType.bypass,
                replica_groups=[list(range(NCORES))],
                ins=[v16_dram[:].opt()],
                outs=[vfull_dram[:].opt()],
            )
            v_rep = misc.tile([P, NB * ZD], f32)
            nc.gpsimd.dma_start(v_rep[:, 0:ZD], vfull_dram[:])
            # log-doubling replication of v across the NB batch slots
            w = ZD
            while w < NB * ZD:
                nc.vector.tensor_copy(v_rep[:, w:2 * w], v_rep[:, 0:w])
                w *= 2

            # ---- phase Z: r^T[l, b] = sum_k z[b, l, k] v[l, k]; exp --------
            rT = misc.tile([P, BP], f32)
            z_re = z_p.rearrange("(n b) l k -> n l b k", b=NB)
            for t in range(NZT):
                zt = zpool.tile([P, NB, ZD], f32, tag="z")
                if t < Z_BUFS:
                    # data-dep gate: keep the z stream's first round of DMAs
                    # behind the c phase (reads the last csum column, then the
                    # DMA overwrites it) so csum/v/AllGather fire early
                    nc.vector.tensor_copy(zt[0:1, 0, 0:1],
                                          csum_sb[0:1, LP * 2 - 1:LP * 2])
                zd = nc.sync.dma_start(zt[:], z_re[t])
                zflat = zt[:].rearrange("p b k -> p (b k)")
                mul_eng = nc.gpsimd if t in (4, 9, 14) else nc.vector
                mul_eng.tensor_mul(zflat, zflat, v_rep[:])
                nc.vector.reduce_sum(rT[:, t * NB:(t + 1) * NB], zt[:],
                                     axis=mybir.AxisListType.X)

            out_sb = misc.tile([P, BP], f32)
            nc.scalar.activation(out_sb[:], rT[:],
                                 mybir.ActivationFunctionType.Exp)
            nc.sync.dma_start(out_p[:], out_sb[:])

    nc.compile()
    return nc


def get_program():
    global _PROGRAM
    if _PROGRAM is None:
        _PROGRAM = _build_program()
    return _PROGRAM


def shard_inputs(z, c, trans):
    z = np.ascontiguousarray(z, dtype=np.float32)
    c = np.ascontiguousarray(c, dtype=np.float32)
    trans = np.ascontiguousarray(trans, dtype=np.float32)
    in_maps = []
    for i in range(NCORES):
        ls = slice(i * LP, (i + 1) * LP)
        in_maps.append({
            "z": z[i * BP:(i + 1) * BP],
            "ct": np.ascontiguousarray(c[:, ls, :].transpose(1, 2, 0)),
            "tt": np.ascontiguousarray(trans[ls].transpose(0, 2, 1)),
        })
    return in_maps


def gather_output(results):
    out = np.empty((B, L), np.float32)
    for i in range(NCORES):
        out[i * BP:(i + 1) * BP] = results[i]["out"].T
    return out


def kernel(z, c, trans):
    from concourse.bass_utils import run_bass_kernel_spmd

    nc = get_program()
    in_maps = shard_inputs(z, c, trans)
    res = run_bass_kernel_spmd(nc, in_maps, list(range(NCORES)))
    return gather_output(res.results)


# revision 15
# speedup vs baseline: 1.0944x; 1.0944x over previous
"""Trainium2 Bass kernel: density-ratio estimator loss.

Math (from the reference):
    csum = sum_b c[b, l, :]                  # (L, C)
    v[l, :] = trans[l] @ csum[l]             # (L, Z)
    r[b, l] = z[b, l, :] . v[l, :]           # (B, L)
    out = exp(r)

Sharding across 8 NeuronCores (full inputs in, full output out):
    - c     : sharded along L (16 steps per core), host-transposed to
              [l, c, b] so csum is a free-axis reduce on the Scalar (ACT)
              engine yielding column-oriented csum directly.
    - trans : sharded along L, host-pre-transposed to [l, c, z] so PE
              matmuls produce v rows (l, z) directly.
    - v     : one tiny AllGather (16x256 f32 per rank -> 128x256).
    - z     : sharded along batch, host-transposed to [l, b, k] so tiles
              are (128 l-partitions, 16 b, 256 k) with 16 KB DMA runs;
              out shard is r^T (L, B/8).

Schedule: c (+tt) DMAs strictly before z DMAs so csum/v/AllGather fire as
early as possible (v matmuls overlap the c phase). z tiles that stream
before v arrives take the DVE multiply path; later tiles are pre-filled
with the v pattern by ACT and multiplied inside the DMA engine
(SWDGE accum_op=mult), leaving DVE only the segmented reductions.
"""

import numpy as np

B, L, ZD, CD = 2048, 128, 256, 256
NCORES = 8
BP = B // NCORES  # 256 batches per core
LP = L // NCORES  # 16 steps per core
P = 128  # SBUF partitions

NB = 16  # batches per z tile
NZT = BP // NB  # 16 z tiles
N_PLAIN = 6  # z tiles on the DVE-multiply path (stream during the AG wait)
C_BUFS = 2
Z_BUFS = 6

_PROGRAM = None


def _build_program():
    import concourse.bacc as bacc
    import concourse.mybir as mybir
    import concourse.tile as tile

    f32 = mybir.dt.float32
    nc = bacc.Bacc("TRN2", target_bir_lowering=False, debug=False,
                   num_devices=NCORES)

    z_p = nc.dram_tensor("z", [L, BP, ZD], f32, kind="ExternalInput").ap()
    ct_p = nc.dram_tensor("ct", [LP, CD, B], f32, kind="ExternalInput").ap()
    tt_p = nc.dram_tensor("tt", [LP, CD, ZD], f32, kind="ExternalInput").ap()
    out_p = nc.dram_tensor("out", [L, BP], f32, kind="ExternalOutput").ap()

    with tile.TileContext(nc) as tc:
        with (
            tc.tile_pool(name="cpool", bufs=C_BUFS) as cpool,
            tc.tile_pool(name="zpool", bufs=Z_BUFS) as zpool,
            tc.tile_pool(name="dummy", bufs=1) as dummy,
            tc.tile_pool(name="misc", bufs=1) as misc,
            tc.tile_pool(name="psum", bufs=1, space="PSUM") as psum,
            tc.tile_pool(name="dram", bufs=1, space="DRAM") as dram,
        ):
            # warmup collective: wake ncfw early so the real AllGather's
            # trigger latency is paid during the c phase
            wu_sb = misc.tile([1, 16], f32)
            wu_in = dram.tile([1, 16], f32)
            wu_out = dram.tile([NCORES, 16], f32)
            nc.gpsimd.memset(wu_sb[:], 0.0)
            nc.gpsimd.dma_start(wu_in[:], wu_sb[:])
            nc.gpsimd.collective_compute(
                "AllGather", mybir.AluOpType.bypass,
                replica_groups=[list(range(NCORES))],
                ins=[wu_in[:].opt()], outs=[wu_out[:].opt()],
            )

            # ---- phase C: csum columns via ACT free-axis reduction ---------
            # ct[l, c, b]; one contiguous 2 MB DMA per l; accum per c-half
            csum_sb = misc.tile([P, LP * 2], f32)
            tt_sb = misc.tile([P, LP, 2, ZD], f32)
            dm = dummy.tile([P, B], f32)
            for l in range(LP):
                ctile = cpool.tile([P, 2, B], f32, tag="c")
                ring = nc.sync if l % 2 == 0 else nc.scalar
                ring.dma_start(ctile[:], ct_p[l].rearrange("(h p) b -> p h b", h=2))
                for h in range(2):
                    nc.scalar.activation(dm[:], ctile[:, h, :],
                                         mybir.ActivationFunctionType.Copy,
                                         accum_out=csum_sb[:, 2 * l + h:2 * l + h + 1])
                if l == 1:
                    # transT early: v matmuls can then overlap the c phase
                    nc.sync.dma_start(
                        tt_sb[:],
                        tt_p.rearrange("l (h p) z -> p l h z", h=2))

            # ---- v rows: v[l, z] = sum_c csum[l, c] * transT[l, c, z] ------
            # PE out must start at partition 0 -> accumulate all v as one row
            pv = psum.tile([1, LP * ZD], f32, tag="ps")
            for l in range(LP):
                for h in range(2):
                    nc.tensor.matmul(
                        pv[0:1, l * ZD:(l + 1) * ZD],
                        csum_sb[:, 2 * l + h: 2 * l + h + 1],
                        tt_sb[:, l, h, :],
                        start=(h == 0), stop=(h == 1),
                    )
            # reuse tt_sb row 0 as the v16 staging row (tt is dead after pv)
            v16_sb = tt_sb[:].rearrange("p a b c -> p (a b c)")[0:1, 0:LP * ZD]
            nc.scalar.copy(v16_sb, pv[:])

            # ---- AllGather v -----------------------------------------------
            v16_dram = dram.tile([1, LP * ZD], f32)
            vfull_dram = dram.tile([L, ZD], f32)
            nc.scalar.dma_start(v16_dram[:], v16_sb)
            nc.gpsimd.collective_compute(
                "AllGather",
                mybir.AluOpType.bypass,
                replica_groups=[list(range(NCORES))],
                ins=[v16_dram[:].opt()],
                outs=[vfull_dram[:].opt()],
            )
            v_rep = misc.tile([P, NB * ZD], f32)
            nc.gpsimd.dma_start(v_rep[:, 0:ZD], vfull_dram[:])
            # log-doubling replication of v across the NB batch slots
            w = ZD
            while w < NB * ZD:
                nc.vector.tensor_copy(v_rep[:, w:2 * w], v_rep[:, 0:w])
                w *= 2

            # ---- phase Z: r^T[l, b] = sum_k z[l, b, k] v[l, k]; exp --------
            rT = misc.tile([P, BP], f32)
            for t in range(NZT):
                zt = zpool.tile([P, NB, ZD], f32, tag="z")
                zflat = zt[:].rearrange("p b k -> p (b k)")
                if t < N_PLAIN:
                    # data-dep gate: keep this DMA behind the c phase (reads
                    # the last csum column, then the DMA overwrites it)
                    nc.vector.tensor_copy(zt[0:1, 0, 0:1],
                                          csum_sb[0:1, LP * 2 - 1:LP * 2])
                nc.sync.dma_start(zt[:], z_p[:, t * NB:(t + 1) * NB, :])
                nc.vector.tensor_mul(zflat, zflat, v_rep[:])
                rsl = rT[:, t * NB:(t + 1) * NB]
                if t % 2 == 0:
                    # ACT-side segmented reduce: per-b accumulate
                    for b in range(NB):
                        nc.scalar.activation(
                            dm[:, 0:ZD], zt[:, b, :],
                            mybir.ActivationFunctionType.Copy,
                            accum_out=rsl[:, b:b + 1])
                else:
                    nc.vector.reduce_sum(rsl, zt[:],
                                         axis=mybir.AxisListType.X)

            out_sb = misc.tile([P, BP], f32)
            nc.scalar.activation(out_sb[:], rT[:],
                                 mybir.ActivationFunctionType.Exp)
            nc.sync.dma_start(out_p[:], out_sb[:])

    nc.compile()
    return nc


def get_program():
    global _PROGRAM
    if _PROGRAM is None:
        _PROGRAM = _build_program()
    return _PROGRAM


def shard_inputs(z, c, trans):
    z = np.ascontiguousarray(z, dtype=np.float32)
    c = np.ascontiguousarray(c, dtype=np.float32)
    trans = np.ascontiguousarray(trans, dtype=np.float32)
    in_maps = []
    for i in range(NCORES):
        ls = slice(i * LP, (i + 1) * LP)
        in_maps.append({
            "z": np.ascontiguousarray(
                z[i * BP:(i + 1) * BP].transpose(1, 0, 2)),
            "ct": np.ascontiguousarray(c[:, ls, :].transpose(1, 2, 0)),
            "tt": np.ascontiguousarray(trans[ls].transpose(0, 2, 1)),
        })
    return in_maps


def gather_output(results):
    out = np.empty((B, L), np.float32)
    for i in range(NCORES):
        out[i * BP:(i + 1) * BP] = results[i]["out"].T
    return out


def kernel(z, c, trans):
    from concourse.bass_utils import run_bass_kernel_spmd

    nc = get_program()
    in_maps = shard_inputs(z, c, trans)
    res = run_bass_kernel_spmd(nc, in_maps, list(range(NCORES)))
    return gather_output(res.results)
